# revision 18
# baseline (speedup 1.0000x reference)
"""GAT (2-layer graph attention) Trainium2 Bass kernel, 8-core row-parallel.

Strategy
--------
Shard the destination-node dimension N=8192 across 8 cores (1024 rows each).
Attention tiles are computed TRANSPOSED, [j=128 partitions, i=1024 free], so:
  * f2[j] rides as the per-partition scalar of a fused custom DVE op,
  * the PV matmul needs no on-chip transposes of the attention matrix
    (lhsT = [ones | fts] is stationary, exp-tile is the moving operand),
  * the softmax row-sum falls out of the same matmul via the ones column.
Per (j-chunk, head) unit the dense work is exactly:
  1 custom DVE op   lb = max(l, alpha*l) + biasT   (l = f1b + f2 + B)
  1 ScalarE Exp     E  = exp(lb)          (masked entries underflow to 0.0)
  2 TensorE matmuls acc[:, i] += [1|fts]^T @ E     (float32r, N=512 each)
The bias matrix is read once per layer per core: cast to bf16 by a SWDGE
DMA and transposed on-chip by the DMA xbar (2-byte transpose path). The
bf16 cast only changes -1e9 to -9.98e8, which exp() maps to 0.0 either way;
on-edge zeros stay exact.
Layer-1 output (h1^T, already in the [feature, node] layout layer 2 wants as
its lhsT) is gathered on the host between the two launches.
"""

import sys

if "/opt/trn_rl_repo" not in sys.path:
    sys.path.insert(0, "/opt/trn_rl_repo")

from contextlib import ExitStack

import ml_dtypes
import numpy as np

import concourse.bass as bass
import concourse.mybir as mybir
import concourse.tile as tile
from concourse import bacc, bass_utils
from concourse import dve_ops
from concourse.dve_ops import DveOp
from concourse.dve_spec import (
    C0,
    C1,
    C2,
    Spec,
    Src0,
    Src1,
    _has_src1,
    lower,
    maxx,
)
from concourse.dve_uop import DveOpSpec

F32 = mybir.dt.float32
F32R = mybir.dt.float32r
BF16 = mybir.dt.bfloat16
EXP = mybir.ActivationFunctionType.Exp
ADD = mybir.AluOpType.add
MIN = mybir.AluOpType.min
MAX = mybir.AluOpType.max
MULT = mybir.AluOpType.mult

N, F_IN, HID, NH1, NCLS = 8192, 256, 64, 4, 16
NCORES = 8
ROWS = N // NCORES  # 1024 destination rows per core
NCHUNK = N // 128  # 64 source-node chunks
ALPHA = 0.2

# Set False if mixed-dtype (f32 in0 / bf16 in1) custom-DVE turns out to work;
# True inserts an explicit bf16->f32 convert of the transposed bias tile.
CONVERT_MASK = True


def _register_lrelu_op() -> DveOp:
    """out = max(x, alpha*x) + in1,  x = in0 + s0 + imm2.

    in0: broadcast f1 tile [P, N];  in1: transposed bias tile [P, N];
    s0: per-partition f2 column [P, 1];  s1(C1): alpha;  imm2: b_src+b_dst.
    """
    name = "LRELU_BIAS_ADD_GAT"
    if name in dve_ops._SUB_OPCODE_FOR_NAME:
        return next(o for o in dve_ops.OPS if o.name == name)
    x = Src0 + C0
    body = maxx(x, x * C1) + Src1

    def _ref(in0, in1, s0, s1, imm2):
        l = in0.astype(np.float32) + s0
        return np.maximum(l, l * s1) + in1.astype(np.float32)

    spec = Spec(body=body, reference=_ref)
    row = dve_ops._CUSTOM_DVE_ROW_BASE + len(dve_ops.OPS)
    assert row < 0x20
    shas = {}
    for ver in ("v3", "v4"):
        u = lower(spec, ver=ver)
        shas[ver] = DveOpSpec(
            name=name, opcode=row, uops=u, rd1_en=_has_src1(spec)
        ).sha(ver)
    op = DveOp(name, spec, subdim=False, uops_sha=shas)
    dve_ops.OPS.append(op)
    dve_ops._SUB_OPCODE_FOR_NAME[name] = row
    dve_ops.CUSTOM_DVE_SPECS[name] = spec
    return op


def _build_layer(nh, hid, fcat, Bvals, alpha, elu):
    """Build one SPMD launch (layer). Returns (nc, in_names, out_names).

    nh:   number of heads (4 or 1)
    hid:  per-head output width (64 or 16)
    fcat: input feature dim (256 both layers)
    Bvals: per-head b_src+b_dst floats, baked as imm2
    elu:  apply ELU activation to the normalized output
    """
    op = _register_lrelu_op()
    ata = next(o for o in dve_ops.OPS if o.name == "AFFINE_THEN_ADD")
    nc = bacc.Bacc("TRN2", target_bir_lowering=False, debug=False, num_devices=1)
    kc = fcat // 128  # contraction chunks for the feature matmul
    nhp = max(nh, 4)  # f32r matmul needs moving free dim > 1; pad Wsrc
    wid = hid + 2  # [W | W@a_src | W@a_dst] per head
    blk = nh * (hid + 1)  # per-chunk fts storage: nh * [ones | fts]

    # DRAM I/O ------------------------------------------------------------
    xT = nc.dram_tensor("xT", [fcat, N], F32R, kind="ExternalInput").ap()
    own_xT = nc.dram_tensor("own_xT", [fcat, ROWS], F32R, kind="ExternalInput").ap()
    Wcat = nc.dram_tensor("Wcat", [fcat, nh * wid], F32R, kind="ExternalInput").ap()
    Wsrc = nc.dram_tensor("Wsrc", [fcat, nhp], F32R, kind="ExternalInput").ap()
    bias_out = nc.dram_tensor("bias_out", [nh, hid], F32, kind="ExternalInput").ap()
    if elu:  # layer 1: raw f32 bias rows in, bf16 transposed bias out
        bias_rows = nc.dram_tensor("bias_rows", [ROWS, N], F32, kind="ExternalInput").ap()
        mbT_out = nc.dram_tensor("mbT", [N, ROWS], BF16, kind="ExternalOutput").ap()
    else:  # layer 2: pre-transposed bf16 bias in
        mbT_in = nc.dram_tensor("mbT_in", [N, ROWS], BF16, kind="ExternalInput").ap()
    outT = nc.dram_tensor("outT", [nh * hid, ROWS], F32, kind="ExternalOutput").ap()

    with tile.TileContext(nc) as tc, ExitStack() as ctx:
        const = ctx.enter_context(tc.sbuf_pool(name="const", bufs=1))

        # resident constants -------------------------------------------------
        w_t = []
        for k in range(kc):
            wk = const.tile([128, nh * wid], F32R, tag=f"w{k}", name=f"w{k}")
            nc.sync.dma_start(wk, Wcat[k * 128 : (k + 1) * 128, :])
            w_t.append(wk)
        ws_t = []
        for k in range(kc):
            wsk = const.tile([128, nhp], F32R, tag=f"ws{k}", name=f"ws{k}")
            nc.sync.dma_start(wsk, Wsrc[k * 128 : (k + 1) * 128, :])
            ws_t.append(wsk)
        cst = const.tile([1, 128 + hid + 1], F32, tag="cst", name="cst")
        nc.gpsimd.memset(cst, 1.0)
        nc.gpsimd.memset(cst[:, 128 : 129], 0.0)
        onesrow = const.tile([1, 128], F32R, tag="onesrow", name="onesrow")
        nc.vector.tensor_copy(onesrow, cst[:, 0:128])
        # lhsT for the reciprocal broadcast: [0, 1, 1, ..., 1]
        maskh = const.tile([1, hid + 1], F32R, tag="maskh", name="maskh")
        nc.vector.tensor_copy(maskh, cst[:, 128 : 128 + hid + 1])
        ident = const.tile([128, 128], F32, tag="ident", name="ident")
        from concourse.masks import make_identity

        make_identity(nc, ident)
        bpp = []
        for h in range(nh):
            bt = const.tile([hid + 1, 1], F32, tag=f"bpp{h}", name=f"bpp{h}")
            nc.gpsimd.memset(bt, 0.0)
            nc.sync.dma_start(
                bt[1 : hid + 1, :], bias_out[h : h + 1, :].rearrange("a b -> b a")
            )
            bpp.append(bt)

        ones128 = const.tile([128, 4], F32, tag="ones128", name="ones128")
        nc.gpsimd.memset(ones128, 1.0)
        fts_all = const.tile([128, NCHUNK * blk], F32R, tag="fts", name="fts_all")
        f2_all = const.tile([128, NCHUNK * nh], F32, tag="f2", name="f2_all")
        f1col = const.tile([128, 8 * nhp], F32, tag="f1col", name="f1col")
        f1row = []
        f1b = []
        for h in range(nh):
            fr = const.tile([1, ROWS], F32R, tag=f"f1row{h}", name=f"f1row{h}")
            f1row.append(fr)
            fb = const.tile([128, ROWS], F32, tag=f"f1b{h}", name=f"f1b{h}")
            f1b.append(fb)

        # ---- prologue: fts = x @ Wcat for all j; f1 for own rows ----------
        with tc.psum_pool(name="pro", bufs=4) as pp, tc.sbuf_pool(
            name="pro_sb", bufs=4
        ) as ps:
            for jc in range(NCHUNK):
                sq = []
                for k in range(kc):
                    s = ps.tile([128, 128], F32R, tag=f"sq{k}", name=f"sq{k}")
                    nc.sync.dma_start(
                        s, xT[k * 128 : (k + 1) * 128, jc * 128 : (jc + 1) * 128]
                    )
                    sq.append(s)
                pf = pp.tile([128, nh * wid], F32, tag="ps", name="pf")
                for k in range(kc):
                    nc.tensor.matmul(
                        pf,
                        lhsT=sq[k],
                        rhs=w_t[k],
                        start=(k == 0),
                        stop=(k == kc - 1),
                    )
                # fts columns: psum [h, 0:hid] -> fts_all [jc, h, 1:hid+1]
                src = pf.rearrange("p (h x) -> p h x", h=nh)[:, :, 0:hid]
                dst = fts_all.rearrange("p (c h x) -> p c h x", c=NCHUNK, h=nh)[
                    :, jc, :, 1 : hid + 1
                ]
                nc.vector.tensor_copy(dst, src)
                # f2 (dst-attention) columns: psum [h, hid+1]
                nc.vector.tensor_copy(
                    f2_all.rearrange("p (c h) -> p c h", c=NCHUNK)[:, jc, :],
                    pf.rearrange("p (h x) -> p h x", h=nh)[:, :, hid + 1 : hid + 2],
                )
                nc.vector.tensor_copy(
                    fts_all.rearrange("p (a x) -> p a x", x=hid + 1)[
                        :, jc * nh : (jc + 1) * nh, 0
                    ],
                    ones128[:, 0:nh],
                )
            # own-row f1 columns (src-attention term for this core's rows)
            for r in range(8):
                so = []
                for k in range(kc):
                    s = ps.tile([128, 128], F32R, tag=f"so{k}", name=f"so{k}")
                    nc.sync.dma_start(
                        s, own_xT[k * 128 : (k + 1) * 128, r * 128 : (r + 1) * 128]
                    )
                    so.append(s)
                pf1 = pp.tile([128, nhp], F32, tag="ps", name="pf1")
                for k in range(kc):
                    nc.tensor.matmul(
                        pf1,
                        lhsT=so[k],
                        rhs=ws_t[k],
                        start=(k == 0),
                        stop=(k == kc - 1),
                    )
                nc.vector.tensor_copy(f1col[:, r * nhp : (r + 1) * nhp], pf1)
            # f1row[h] via per-column PE transposes, then broadcast via PE
            for r in range(8):
                for h in range(nh):
                    pt = pp.tile([1, 128], F32, tag="ps", name="pt")
                    nc.tensor.transpose(
                        pt, f1col[:, r * nhp + h : r * nhp + h + 1], ident
                    )
                    nc.vector.tensor_copy(
                        f1row[h][:, r * 128 : (r + 1) * 128], pt
                    )
            for h in range(nh):
                # fold the (b_src + b_dst) scalar into f1 here
                if Bvals[h] != 0.0:
                    nc.vector.tensor_scalar_add(f1row[h], f1row[h], Bvals[h])
                for half in range(ROWS // 512):
                    pb = pp.tile([128, 512], F32, tag="ps", name="pb")
                    nc.tensor.matmul(
                        pb,
                        lhsT=onesrow,
                        rhs=f1row[h][:, half * 512 : (half + 1) * 512],
                        start=True,
                        stop=True,
                    )
                    nc.vector.tensor_copy(
                        f1b[h][:, half * 512 : (half + 1) * 512], pb
                    )

        # ---- attention sweep over source chunks ---------------------------
        with tc.psum_pool(name="acc", bufs=8) as ap_, tc.sbuf_pool(
            name="sw", bufs=3
        ) as sw, tc.sbuf_pool(name="ep", bufs=1) as ep:
            accs = []
            for i in range(2 * nh):
                a = ap_.tile([hid + 1, 512], F32, tag=f"acc{i}", name=f"acc{i}", bufs=1)
                accs.append(a)
            for jc in range(NCHUNK):
                mt = sw.tile([128, ROWS], BF16, tag="mt", name="mt")
                if elu:
                    mn = sw.tile([128, 8, 128], BF16, tag="mn", name="mn")
                    nc.gpsimd.dma_start(
                        mn,
                        bias_rows.rearrange("(ic p) J -> p ic J", p=128)[
                            :, :, jc * 128 : (jc + 1) * 128
                        ],
                    )
                    for ic in range(8):
                        nc.sync.dma_start_transpose(
                            mt[:, ic * 128 : (ic + 1) * 128], mn[:, ic, :]
                        )
                    nc.sync.dma_start(mbT_out[jc * 128 : (jc + 1) * 128, :], mt)
                else:
                    nc.sync.dma_start(mt, mbT_in[jc * 128 : (jc + 1) * 128, :])
                if CONVERT_MASK:
                    mtf = sw.tile([128, ROWS], F32, tag="mtf", name="mtf")
                    nc.vector.tensor_copy(mtf, mt)
                else:
                    mtf = mt
                for h in range(nh):
                    lb = sw.tile([128, ROWS], F32, tag="lb", name="lb")
                    nc.vector._custom_dve(
                        op,
                        out=lb,
                        in0=f1b[h],
                        in1=mtf,
                        s0=f2_all[:, jc * nh + h : jc * nh + h + 1],
                        s1=alpha,
                    )
                    e_t = sw.tile([128, ROWS], F32R, tag="e_t", name="e_t")
                    nc.scalar.activation(e_t, lb, EXP)
                    lhs = fts_all.rearrange(
                        "p (c h x) -> p c h x", c=NCHUNK, h=nh
                    )[:, jc, h, :]
                    for half in range(2):
                        nc.tensor.matmul(
                            accs[2 * h + half],
                            lhsT=lhs,
                            rhs=e_t[:, half * 512 : (half + 1) * 512],
                            start=(jc == 0),
                            stop=(jc == NCHUNK - 1),
                        )

            # ---- epilogue: normalize (+bias, +ELU), store h^T -------------
            for h in range(nh):
                v = ep.tile([hid + 1, ROWS], F32, tag="v", name="v")
                for half in range(2):
                    nc.vector.tensor_copy(
                        v[:, half * 512 : (half + 1) * 512], accs[2 * h + half]
                    )
                rc = ep.tile([1, ROWS], F32R, tag="rc", name="rc")
                with nc.allow_low_precision(reason="f32r out of reciprocal"):
                    nc.vector.reciprocal(rc, v[0:1, :])
                t = ep.tile([hid + 1, ROWS], F32, tag="t", name="t")
                for half in range(2):
                    pb2 = ap_.tile(
                        [hid + 1, 512], F32, tag=f"acc{2 * h + half}", name="pb2", bufs=1
                    )
                    nc.tensor.matmul(
                        pb2,
                        lhsT=maskh,
                        rhs=rc[:, half * 512 : (half + 1) * 512],
                        start=True,
                        stop=True,
                    )
                    nc.vector.tensor_tensor(
                        t[:, half * 512 : (half + 1) * 512],
                        v[:, half * 512 : (half + 1) * 512],
                        pb2,
                        op=MULT,
                    )
                # row 0 carries harmless junk through the tail ops
                if elu:
                    m_ = ep.tile([hid + 1, ROWS], F32, tag="m_", name="m_")
                    nc.vector.tensor_scalar(
                        m_, t, bpp[h], 0.0, op0=ADD, op1=MIN
                    )
                    r_ = ep.tile([hid + 1, ROWS], F32, tag="r_", name="r_")
                    nc.vector.tensor_scalar(
                        r_, t, bpp[h], 0.0, op0=ADD, op1=MAX
                    )
                    e2 = ep.tile([hid + 1, ROWS], F32, tag="e2", name="e2")
                    nc.scalar.activation(e2, m_, EXP)
                    o_ = ep.tile([hid + 1, ROWS], F32, tag="o_", name="o_")
                    nc.vector._custom_dve(
                        ata,
                        out=o_,
                        in0=e2,
                        in1=r_,
                        s0=1.0,
                        s1=-1.0,
                    )
                else:
                    o_ = ep.tile([hid + 1, ROWS], F32, tag="o_", name="o_")
                    nc.vector.tensor_scalar(
                        o_, t, bpp[h], None, op0=ADD
                    )
                nc.sync.dma_start(outT[h * hid : (h + 1) * hid, :], o_[1 : hid + 1, :])

    nc.compile()
    return nc


_BUILD_CACHE: dict = {}


def _get_layer(key, *args):
    if key not in _BUILD_CACHE:
        _BUILD_CACHE[key] = _build_layer(*args)
    return _BUILD_CACHE[key]


def kernel(
    seq,
    bias_mat,
    W1,
    a1_src,
    a1_dst,
    b1_src,
    b1_dst,
    bias1,
    W2,
    a2_src,
    a2_dst,
    b2_src,
    b2_dst,
    bias2,
):
    seq = np.asarray(seq, np.float32)
    bias_mat = np.asarray(bias_mat, np.float32)
    W1, W2 = np.asarray(W1, np.float32), np.asarray(W2, np.float32)
    a1_src, a1_dst = np.asarray(a1_src, np.float32), np.asarray(a1_dst, np.float32)
    a2_src, a2_dst = np.asarray(a2_src, np.float32), np.asarray(a2_dst, np.float32)
    bias1, bias2 = np.asarray(bias1, np.float32), np.asarray(bias2, np.float32)

    x = seq[0]  # [N, F_IN]
    xT = np.ascontiguousarray(x.T)  # [F_IN, N]
    W1cat = np.concatenate(
        [np.concatenate([W1[h], W1[h] @ a1_src[h], W1[h] @ a1_dst[h]], axis=1)
         for h in range(NH1)],
        axis=1,
    )  # [256, 4*66]
    W1s = np.concatenate([W1[h] @ a1_src[h] for h in range(NH1)], axis=1)  # [256, 4]
    B1 = tuple(float(b1_src[h, 0] + b1_dst[h, 0]) for h in range(NH1))

    nc1 = _get_layer(("L1", B1), NH1, HID, F_IN, B1, ALPHA, True)
    in_maps = []
    for c in range(NCORES):
        in_maps.append(
            {
                "xT": xT,
                "own_xT": np.ascontiguousarray(xT[:, c * ROWS : (c + 1) * ROWS]),
                "Wcat": W1cat,
                "Wsrc": W1s,
                "bias_out": bias1,
                "bias_rows": np.ascontiguousarray(
                    bias_mat[0, c * ROWS : (c + 1) * ROWS, :]
                ),
            }
        )
    res1 = bass_utils.run_bass_kernel_spmd(nc1, in_maps, core_ids=list(range(NCORES)))
    h1T = np.concatenate([r["outT"] for r in res1.results], axis=1)  # [256, 8192]
    mbTs = [r["mbT"] for r in res1.results]  # per-core [8192, 1024] bf16

    W2cat = np.concatenate([W2[0], W2[0] @ a2_src[0], W2[0] @ a2_dst[0]], axis=1)
    W2s = np.concatenate([W2[0] @ a2_src[0], np.zeros((NH1 * HID, 3), np.float32)], axis=1)  # [256, 4] padded
    B2 = (float(b2_src[0, 0] + b2_dst[0, 0]),)

    nc2 = _get_layer(("L2", B2), 1, NCLS, NH1 * HID, B2, ALPHA, False)
    in_maps2 = []
    for c in range(NCORES):
        in_maps2.append(
            {
                "xT": h1T,
                "own_xT": np.ascontiguousarray(h1T[:, c * ROWS : (c + 1) * ROWS]),
                "Wcat": W2cat,
                "Wsrc": W2s,
                "bias_out": bias2,
                "mbT_in": np.asarray(mbTs[c], ml_dtypes.bfloat16),
            }
        )
    res2 = bass_utils.run_bass_kernel_spmd(nc2, in_maps2, core_ids=list(range(NCORES)))
    outT = np.concatenate([r["outT"] for r in res2.results], axis=1)  # [16, 8192]
    return np.ascontiguousarray(outT.T)[None].astype(np.float32)  # [1, 8192, 16]
